# revision 7
# baseline (speedup 1.0000x reference)
"""Trainium2 Bass kernel for a dense pre-LN decoder layer (MHA + FFN).

Sharding (8 NeuronCores, one chip):
  - Attention: tensor-parallel over heads (16 heads -> 2 per core).
  - FFN: tensor-parallel over the 4*d_model hidden dim (8192 -> 1024 per core).
  - LayerNorms: sequence-parallel (4096 tokens -> 512 per core) with
    AllGather of the normalized activations (bf16).
  - Attention output partials: ReduceScatter over tokens; final FFN partials
    are summed on the host (the gather/unshard step).

All matmuls run in bf16 with fp32 PSUM accumulation. LN gains are folded
into the following weight matrices on the host; LN biases fold into
per-output-channel biases; 1/sqrt(head_dim) folds into W_q; the causal mask
is pre-added into the alibi bias on the host.
"""

import math
import sys

import numpy as np

sys.path.insert(0, "/opt/trn_rl_repo")

import concourse.bass as bass  # noqa: E402
import concourse.tile as tile  # noqa: E402
from concourse import bacc, mybir  # noqa: E402
from concourse.bass_utils import run_bass_kernel_spmd  # noqa: E402

try:
    from ml_dtypes import bfloat16 as np_bf16
except ImportError:  # pragma: no cover
    import jax.numpy as jnp

    np_bf16 = jnp.bfloat16

# ---------------------------------------------------------------- constants
NCORES = 8
D = 2048          # d_model
S = 2048          # sequence length
B = 2             # batch
NTOK = B * S      # 4096 global tokens
HD = 128          # head dim
NH = 16           # total heads
HPC = NH // NCORES      # heads per core = 2
DFF = 4 * D             # 8192
DFFC = DFF // NCORES    # ffn hidden per core = 1024
TOKC = NTOK // NCORES   # tokens per core for sequence-parallel = 512
LN_EPS = 1e-5
NEG = -1.0e30

P = 128           # SBUF partitions
SPAN = 512        # token span for matmul rhs
NSPAN = NTOK // SPAN        # 8
NQB = S // P                # 16 q blocks per batch element
KCH = D // P                # 16 contraction chunks of 128 over d_model
FSL = DFFC // P             # 8 ffn slices of 128
F32 = mybir.dt.float32
BF16 = mybir.dt.bfloat16

_CACHE = {}
LAST_RESULT = None


# ---------------------------------------------------------------- program
def build_program():
    nc = bacc.Bacc(
        "TRN2", target_bir_lowering=False, debug=False, num_devices=NCORES
    )

    # -------- per-core I/O (same shapes on every core; data differs)
    xc = nc.dram_tensor("xc", [TOKC, D], F32, kind="ExternalInput").ap()
    alibi = nc.dram_tensor("alibi", [HPC, S, S], F32, kind="ExternalInput").ap()
    wq = nc.dram_tensor("wq", [D, HPC * HD], BF16, kind="ExternalInput").ap()
    wk = nc.dram_tensor("wk", [D, HPC * HD], BF16, kind="ExternalInput").ap()
    wv = nc.dram_tensor("wv", [D, HPC * HD], BF16, kind="ExternalInput").ap()
    bqkv = nc.dram_tensor("bqkv", [3 * HPC * HD], F32, kind="ExternalInput").ap()
    wo = nc.dram_tensor("wo", [HPC * HD, D], BF16, kind="ExternalInput").ap()
    w1 = nc.dram_tensor("w1", [D, DFFC], BF16, kind="ExternalInput").ap()
    b1 = nc.dram_tensor("b1", [DFFC], F32, kind="ExternalInput").ap()
    w2 = nc.dram_tensor("w2", [DFFC, D], BF16, kind="ExternalInput").ap()

    ffn_part = nc.dram_tensor(
        "ffn_part", [NTOK, D], BF16, kind="ExternalOutput"
    ).ap()
    h_part = nc.dram_tensor("h_part", [TOKC, D], F32, kind="ExternalOutput").ap()

    groups = [list(range(NCORES))]

    with tile.TileContext(nc) as tc:
        _build(tc, nc, xc, alibi, wq, wk, wv, bqkv, wo, w1, b1, w2,
               ffn_part, h_part, groups)

    nc.compile()
    return nc


def _layernorm_tiles(nc, pool, src_rows, dst_rows, n_tiles, h_out_rows=None,
                     rs_rows=None, tag=""):
    """LN over [n_tiles*128, D] rows: read f32 rows from `src_rows`
    (callable tile_idx -> DRAM AP), write normalized bf16 rows to
    `dst_rows`. If rs_rows is given, first add it (bf16 residual) to the
    input and also store the f32 sum to h_out_rows."""
    eps_t = pool.tile([P, 1], F32, tag=f"ln_eps{tag}", bufs=1)
    nc.vector.memset(eps_t[:], LN_EPS)
    for t in range(n_tiles):
        x_t = pool.tile([P, D], F32, tag=f"ln_x{tag}", name=f"ln_x{tag}_{t}")
        nc.gpsimd.dma_start(out=x_t[:], in_=src_rows(t))
        if rs_rows is not None:
            rs_t = pool.tile([P, D], BF16, tag=f"ln_rs{tag}",
                             name=f"ln_rs{tag}_{t}")
            nc.gpsimd.dma_start(out=rs_t[:], in_=rs_rows(t))
            nc.vector.tensor_add(out=x_t[:], in0=x_t[:], in1=rs_t[:])
            nc.gpsimd.dma_start(out=h_out_rows(t), in_=x_t[:])
        # stats: 4 chunks of 512 through bn_stats, then aggregate
        x_view = x_t[:].rearrange("p (c f) -> p c f", f=512)
        st = pool.tile([P, 4, 6], F32, tag=f"ln_st{tag}", name=f"ln_st{tag}_{t}")
        for c in range(4):
            nc.vector.bn_stats(out=st[:, c, :], in_=x_view[:, c, :])
        mv = pool.tile([P, 2], F32, tag=f"ln_mv{tag}", name=f"ln_mv{tag}_{t}")
        nc.vector.bn_aggr(out=mv[:], in_=st[:])
        # inv = 1/sqrt(var+eps)
        inv = pool.tile([P, 1], F32, tag=f"ln_inv{tag}", name=f"ln_inv{tag}_{t}")
        nc.scalar.activation(out=inv[:], in_=mv[:, 1:2],
                             func=mybir.ActivationFunctionType.Sqrt,
                             bias=eps_t[:], scale=1.0)
        nc.vector.reciprocal(out=inv[:], in_=inv[:])
        xh = pool.tile([P, D], BF16, tag=f"ln_xh{tag}", name=f"ln_xh{tag}_{t}")
        nc.vector.tensor_scalar(
            out=xh[:], in0=x_t[:], scalar1=mv[:, 0:1], scalar2=inv[:],
            op0=mybir.AluOpType.subtract, op1=mybir.AluOpType.mult)
        nc.gpsimd.dma_start(out=dst_rows(t), in_=xh[:])


def _build(tc, nc, xc, alibi, wq, wk, wv, bqkv, wo, w1, b1, w2,
           ffn_part, h_part, groups):
    import contextlib

    ctx = contextlib.ExitStack()
    with ctx:
        dram = ctx.enter_context(tc.tile_pool(name="dram", bufs=1, space="DRAM"))
        xhat_in = dram.tile([TOKC, D], BF16)
        xhat_ag = dram.tile([NTOK, D], BF16, addr_space="Shared")
        attn_part = dram.tile([NTOK, D], BF16)
        rs_out = dram.tile([TOKC, D], BF16)
        hn_in = dram.tile([TOKC, D], BF16)
        hn_ag = dram.tile([NTOK, D], BF16, addr_space="Shared")

        # persistent sbuf: qT/kT per head, v natural per (head, batch)
        persist = ctx.enter_context(tc.tile_pool(name="persist", bufs=1))
        qT = [persist.tile([P, NTOK], BF16, name=f"qT{h}") for h in range(HPC)]
        kT = [persist.tile([P, NTOK], BF16, name=f"kT{h}") for h in range(HPC)]
        vnat = [[persist.tile([P, NQB, P], BF16, name=f"vnat{h}_{b}")
                 for b in range(B)] for h in range(HPC)]

        # small weights (loaded once, early)
        wpool = ctx.enter_context(tc.tile_pool(name="weights", bufs=1))
        bqkv_sb = wpool.tile([P, 3 * HPC], F32)
        nc.gpsimd.dma_start(out=bqkv_sb[:],
                            in_=bqkv.rearrange("(a p) -> p a", p=P))
        wo_sb = wpool.tile([P, HPC, D], BF16)
        nc.gpsimd.dma_start(out=wo_sb[:], in_=wo.rearrange("(h p) o -> p h o", p=P))

        # ---------------- phase A: LN1 on own token slice, AllGather
        with tc.tile_pool(name="ln1", bufs=2) as ln1p:
            _layernorm_tiles(
                nc, ln1p,
                src_rows=lambda t: xc[t * P:(t + 1) * P, :],
                dst_rows=lambda t: xhat_in[t * P:(t + 1) * P, :],
                n_tiles=TOKC // P, tag="1")
        nc.gpsimd.collective_compute(
            "AllGather", mybir.AluOpType.bypass, replica_groups=groups,
            ins=[xhat_in.opt()], outs=[xhat_ag.opt()])

        # ---------------- phase B: QKV projections (outputs transposed)
        with tc.tile_pool(name="qkvw", bufs=1) as qwp, \
             tc.tile_pool(name="qkv", bufs=2) as qkvp, \
             tc.tile_pool(name="qkv_ps", bufs=4, space="PSUM") as qkvps:
            wq_sb = qwp.tile([P, KCH, HPC * HD], BF16)
            wk_sb = qwp.tile([P, KCH, HPC * HD], BF16)
            wv_sb = qwp.tile([P, KCH, HPC * HD], BF16)
            nc.gpsimd.dma_start(out=wq_sb[:],
                                in_=wq.rearrange("(c p) o -> p c o", p=P))
            nc.gpsimd.dma_start(out=wk_sb[:],
                                in_=wk.rearrange("(c p) o -> p c o", p=P))
            nc.gpsimd.dma_start(out=wv_sb[:],
                                in_=wv.rearrange("(c p) o -> p c o", p=P))
            vT = [qwp.tile([P, NTOK], BF16, name=f"vT{h}") for h in range(HPC)]
            for sp in range(NSPAN):
                xT = qkvp.tile([P, KCH, SPAN], BF16, tag="xT")
                for kc in range(KCH):
                    nc.sync.dma_start(
                        out=xT[:, kc, :],
                        in_=xhat_ag[sp * SPAN:(sp + 1) * SPAN,
                                    kc * P:(kc + 1) * P],
                        transpose=True)
                for wi, (w_sb, outs) in enumerate(
                        ((wq_sb, qT), (wk_sb, kT), (wv_sb, vT))):
                    for h in range(HPC):
                        ps = qkvps.tile([P, SPAN], F32, tag="ps")
                        for kc in range(KCH):
                            nc.tensor.matmul(
                                ps[:], lhsT=w_sb[:, kc, h * HD:(h + 1) * HD],
                                rhs=xT[:, kc, :],
                                start=(kc == 0), stop=(kc == KCH - 1))
                        col = wi * HPC + h
                        nc.scalar.activation(
                            out=outs[h][:, sp * SPAN:(sp + 1) * SPAN], in_=ps[:],
                            func=mybir.ActivationFunctionType.Identity,
                            bias=bqkv_sb[:, col:col + 1], scale=1.0)

            # v natural layout per (h, b): [k-chunk partitions, hd]
            for h in range(HPC):
                for b in range(B):
                    for j in range(NQB):
                        nc.sync.dma_start(
                            out=vnat[h][b][:, j, :],
                            in_=vT[h][:, b * S + j * P:b * S + (j + 1) * P],
                            transpose=True)

        # ---------------- phase C: attention + W_o partials
        with tc.tile_pool(name="att", bufs=3) as ap_, \
             tc.tile_pool(name="att_sm", bufs=4) as smp, \
             tc.tile_pool(name="avt", bufs=8) as avtp, \
             tc.tile_pool(name="att_ps", bufs=2, space="PSUM") as aps, \
             tc.tile_pool(name="av_ps", bufs=2, space="PSUM") as avps, \
             tc.tile_pool(name="wo_ps", bufs=2, space="PSUM") as wops:
            for i in range(NQB):
                ks_n = (i + 4) // 4          # number of 512-wide k slices
                kw = ks_n * SPAN
                avT = [[None] * B for _ in range(HPC)]
                for h in range(HPC):
                    al_t = ap_.tile([P, S], F32, tag="alibi",
                                    name=f"al_{i}_{h}")
                    nc.gpsimd.dma_start(out=al_t[:, :kw],
                                        in_=alibi[h, i * P:(i + 1) * P, 0:kw])
                    for b in range(B):
                        toff = b * S
                        E_t = ap_.tile([P, S], BF16, tag="E",
                                       name=f"E_{i}_{h}_{b}")
                        acc = smp.tile([P, 4], F32, tag="acc",
                                       name=f"acc_{i}_{h}_{b}")
                        for ks in range(ks_n):
                            ps = aps.tile([P, SPAN], F32, tag="scps",
                                          name=f"sc_{i}_{h}_{b}_{ks}")
                            nc.tensor.matmul(
                                ps[:],
                                lhsT=qT[h][:, toff + i * P:toff + (i + 1) * P],
                                rhs=kT[h][:, toff + ks * SPAN:
                                          toff + (ks + 1) * SPAN],
                                start=True, stop=True)
                            s_t = smp.tile([P, SPAN], F32, tag="s",
                                           name=f"s_{i}_{h}_{b}_{ks}")
                            nc.vector.scalar_tensor_tensor(
                                out=s_t[:], in0=ps[:], scalar=1.0,
                                in1=al_t[:, ks * SPAN:(ks + 1) * SPAN],
                                op0=mybir.AluOpType.mult,
                                op1=mybir.AluOpType.add)
                            nc.scalar.activation(
                                out=E_t[:, ks * SPAN:(ks + 1) * SPAN],
                                in_=s_t[:],
                                func=mybir.ActivationFunctionType.Exp,
                                accum_out=acc[:, ks:ks + 1])
                        rec = smp.tile([P, 1], F32, tag="rec",
                                       name=f"rec_{i}_{h}_{b}")
                        nc.vector.reduce_sum(out=rec[:], in_=acc[:, 0:ks_n],
                                             axis=mybir.AxisListType.X)
                        nc.vector.reciprocal(out=rec[:], in_=rec[:])
                        ET_t = ap_.tile([P, NQB, P], BF16, tag="ET",
                                        name=f"ET_{i}_{h}_{b}")
                        for j in range(i + 1):
                            nc.sync.dma_start(
                                out=ET_t[:, j, :],
                                in_=E_t[:, j * P:(j + 1) * P],
                                transpose=True)
                        av_ps = avps.tile([P, P], F32, tag="avps",
                                          name=f"avp_{i}_{h}_{b}")
                        for j in range(i + 1):
                            nc.tensor.matmul(
                                av_ps[:], lhsT=ET_t[:, j, :],
                                rhs=vnat[h][b][:, j, :],
                                start=(j == 0), stop=(j == i))
                        av_sb = smp.tile([P, P], BF16, tag="av",
                                         name=f"av_{i}_{h}_{b}")
                        nc.vector.tensor_scalar_mul(
                            out=av_sb[:], in0=av_ps[:], scalar1=rec[:])
                        avT_t = avtp.tile([P, P], BF16, tag="avT",
                                          name=f"avT_{i}_{h}_{b}")
                        nc.sync.dma_start(out=avT_t[:], in_=av_sb[:],
                                          transpose=True)
                        avT[h][b] = avT_t
                # W_o for this q block (accumulate over local heads)
                for b in range(B):
                    for dsp in range(D // SPAN):
                        ps = wops.tile([P, SPAN], F32, tag="wops",
                                       name=f"wo_{i}_{b}_{dsp}")
                        for h in range(HPC):
                            nc.tensor.matmul(
                                ps[:], lhsT=avT[h][b][:],
                                rhs=wo_sb[:, h, dsp * SPAN:(dsp + 1) * SPAN],
                                start=(h == 0), stop=(h == HPC - 1))
                        o_sb = smp.tile([P, SPAN], BF16, tag="wo_o",
                                        name=f"woo_{i}_{b}_{dsp}")
                        nc.vector.tensor_copy(out=o_sb[:], in_=ps[:])
                        nc.gpsimd.dma_start(
                            out=attn_part[b * S + i * P:b * S + (i + 1) * P,
                                          dsp * SPAN:(dsp + 1) * SPAN],
                            in_=o_sb[:])

        nc.gpsimd.collective_compute(
            "ReduceScatter", mybir.AluOpType.add, replica_groups=groups,
            ins=[attn_part.opt()], outs=[rs_out.opt()])

        # ---------------- phase D: h = x + attn, LN2, AllGather
        with tc.tile_pool(name="ln2", bufs=2) as ln2p:
            _layernorm_tiles(
                nc, ln2p,
                src_rows=lambda t: xc[t * P:(t + 1) * P, :],
                dst_rows=lambda t: hn_in[t * P:(t + 1) * P, :],
                n_tiles=TOKC // P,
                h_out_rows=lambda t: h_part[t * P:(t + 1) * P, :],
                rs_rows=lambda t: rs_out[t * P:(t + 1) * P, :], tag="2")
        nc.gpsimd.collective_compute(
            "AllGather", mybir.AluOpType.bypass, replica_groups=groups,
            ins=[hn_in.opt()], outs=[hn_ag.opt()])

        # ---------------- phase E: FFN
        with tc.tile_pool(name="ffnw", bufs=1) as fwp, \
             tc.tile_pool(name="ffn", bufs=2) as ffnp, \
             tc.tile_pool(name="ffn_ps", bufs=4, space="PSUM") as fps:
            w1_sb = fwp.tile([P, KCH, DFFC], BF16)
            nc.gpsimd.dma_start(out=w1_sb[:],
                                in_=w1.rearrange("(c p) f -> p c f", p=P))
            b1_sb = fwp.tile([P, FSL], F32)
            nc.gpsimd.dma_start(out=b1_sb[:],
                                in_=b1.rearrange("(s p) -> p s", p=P))
            w2_sb = fwp.tile([P, FSL, D], BF16)
            nc.gpsimd.dma_start(out=w2_sb[:],
                                in_=w2.rearrange("(c p) o -> p c o", p=P))
            for sp in range(NSPAN):
                hT = ffnp.tile([P, KCH, SPAN], BF16, tag="hT")
                for kc in range(KCH):
                    nc.sync.dma_start(
                        out=hT[:, kc, :],
                        in_=hn_ag[sp * SPAN:(sp + 1) * SPAN,
                                  kc * P:(kc + 1) * P],
                        transpose=True)
                g1 = ffnp.tile([P, FSL, SPAN], BF16, tag="g1")
                for s in range(FSL):
                    ps = fps.tile([P, SPAN], F32, tag="f1ps")
                    for kc in range(KCH):
                        nc.tensor.matmul(
                            ps[:], lhsT=w1_sb[:, kc, s * P:(s + 1) * P],
                            rhs=hT[:, kc, :],
                            start=(kc == 0), stop=(kc == KCH - 1))
                    nc.scalar.activation(
                        out=g1[:, s, :], in_=ps[:],
                        func=mybir.ActivationFunctionType.Gelu,
                        bias=b1_sb[:, s:s + 1], scale=1.0)
                for tb in range(SPAN // P):
                    for dsp in range(D // SPAN):
                        ps2 = fps.tile([P, SPAN], F32, tag="f2ps")
                        for s in range(FSL):
                            nc.tensor.matmul(
                                ps2[:], lhsT=g1[:, s, tb * P:(tb + 1) * P],
                                rhs=w2_sb[:, s, dsp * SPAN:(dsp + 1) * SPAN],
                                start=(s == 0), stop=(s == FSL - 1))
                        o_sb = ffnp.tile([P, SPAN], BF16, tag="fo")
                        nc.vector.tensor_copy(out=o_sb[:], in_=ps2[:])
                        row = sp * SPAN + tb * P
                        nc.gpsimd.dma_start(
                            out=ffn_part[row:row + P,
                                         dsp * SPAN:(dsp + 1) * SPAN],
                            in_=o_sb[:])


# ---------------------------------------------------------------- host side
def _prep_inputs(x, alibi_bias, W_q, W_k, W_v, W_o, ln1_g, ln1_b, ln2_g,
                 ln2_b, ffn_w1, ffn_b1, ffn_w2, ffn_b2):
    f32 = np.float32
    x = np.ascontiguousarray(np.asarray(x, f32).reshape(NTOK, D))
    inv_sqrt_hd = f32(1.0 / math.sqrt(HD))
    ln1_g = np.asarray(ln1_g, f32)
    ln1_b = np.asarray(ln1_b, f32)
    ln2_g = np.asarray(ln2_g, f32)
    ln2_b = np.asarray(ln2_b, f32)

    wq_f = (ln1_g[:, None] * np.asarray(W_q, f32)) * inv_sqrt_hd
    bq = (ln1_b @ np.asarray(W_q, f32)) * inv_sqrt_hd
    wk_f = ln1_g[:, None] * np.asarray(W_k, f32)
    bk = ln1_b @ np.asarray(W_k, f32)
    wv_f = ln1_g[:, None] * np.asarray(W_v, f32)
    bv = ln1_b @ np.asarray(W_v, f32)
    w1_f = ln2_g[:, None] * np.asarray(ffn_w1, f32)
    b1_f = ln2_b @ np.asarray(ffn_w1, f32) + np.asarray(ffn_b1, f32)

    # alibi with causal mask folded in
    al = np.asarray(alibi_bias, f32).copy()
    iu = np.triu_indices(S, k=1)
    al[:, iu[0], iu[1]] = NEG

    W_o = np.asarray(W_o, f32)
    w2 = np.asarray(ffn_w2, f32)

    in_maps = []
    for c in range(NCORES):
        hs = slice(c * HPC * HD, (c + 1) * HPC * HD)     # head-dim slice
        fs = slice(c * DFFC, (c + 1) * DFFC)             # ffn slice
        ts_ = slice(c * TOKC, (c + 1) * TOKC)            # token slice
        bqkv_c = np.concatenate([bq[hs], bk[hs], bv[hs]]).astype(f32)
        in_maps.append({
            "xc": np.ascontiguousarray(x[ts_]),
            "alibi": np.ascontiguousarray(al[c * HPC:(c + 1) * HPC]),
            "wq": np.ascontiguousarray(wq_f[:, hs].astype(np_bf16)),
            "wk": np.ascontiguousarray(wk_f[:, hs].astype(np_bf16)),
            "wv": np.ascontiguousarray(wv_f[:, hs].astype(np_bf16)),
            "bqkv": bqkv_c,
            "wo": np.ascontiguousarray(W_o[hs, :].astype(np_bf16)),
            "w1": np.ascontiguousarray(w1_f[:, fs].astype(np_bf16)),
            "b1": np.ascontiguousarray(b1_f[fs]),
            "w2": np.ascontiguousarray(w2[fs, :].astype(np_bf16)),
        })
    return in_maps


def kernel(x, alibi_bias, W_q, W_k, W_v, W_o, ln1_g, ln1_b, ln2_g, ln2_b,
           ffn_w1, ffn_b1, ffn_w2, ffn_b2, *, _trace=False, _tmpdir=None):
    global LAST_RESULT
    if "nc" not in _CACHE:
        _CACHE["nc"] = build_program()
    nc = _CACHE["nc"]

    in_maps = _prep_inputs(x, alibi_bias, W_q, W_k, W_v, W_o, ln1_g, ln1_b,
                           ln2_g, ln2_b, ffn_w1, ffn_b1, ffn_w2, ffn_b2)

    res = run_bass_kernel_spmd(
        nc, in_maps, core_ids=list(range(NCORES)),
        trace=_trace, tmpdir=_tmpdir)
    LAST_RESULT = res

    out = np.zeros((NTOK, D), np.float32)
    for c in range(NCORES):
        out += np.asarray(res.results[c]["ffn_part"], np.float32)
    for c in range(NCORES):
        out[c * TOKC:(c + 1) * TOKC] += np.asarray(res.results[c]["h_part"])
    out += np.asarray(ffn_b2, np.float32)[None, :]
    return out.reshape(B, S, D)


# revision 16
# speedup vs baseline: 1.6119x; 1.6119x over previous
"""Trainium2 Bass kernel for a dense pre-LN decoder layer (MHA + FFN).

Sharding (8 NeuronCores, one chip):
  - Attention: tensor-parallel over heads (16 heads -> 2 per core).
  - FFN: tensor-parallel over the 4*d_model hidden dim (8192 -> 1024 per core).
  - LayerNorms: sequence-parallel (4096 tokens -> 512 per core) with
    AllGather of the normalized activations (bf16).
  - Attention output partials: ReduceScatter over tokens; final FFN partials
    are summed on the host (the gather/unshard step).

All matmuls run in bf16 with fp32 PSUM accumulation. LN gains are folded
into the following weight matrices on the host; LN biases fold into
per-output-channel biases; 1/sqrt(head_dim) folds into W_q; the causal mask
is pre-added into the alibi bias on the host.
"""

import math
import sys

import numpy as np

sys.path.insert(0, "/opt/trn_rl_repo")

import concourse.bass as bass  # noqa: E402
import concourse.tile as tile  # noqa: E402
from concourse import bacc, mybir  # noqa: E402
from concourse.bass_utils import run_bass_kernel_spmd  # noqa: E402

try:
    from ml_dtypes import bfloat16 as np_bf16
except ImportError:  # pragma: no cover
    import jax.numpy as jnp

    np_bf16 = jnp.bfloat16

# ---------------------------------------------------------------- constants
NCORES = 8
D = 2048          # d_model
S = 2048          # sequence length
B = 2             # batch
NTOK = B * S      # 4096 global tokens
HD = 128          # head dim
NH = 16           # total heads
HPC = NH // NCORES      # heads per core = 2
DFF = 4 * D             # 8192
DFFC = DFF // NCORES    # ffn hidden per core = 1024
TOKC = NTOK // NCORES   # tokens per core for sequence-parallel = 512
LN_EPS = 1e-5
NEG = -1.0e30

P = 128           # SBUF partitions
SPAN = 512        # token span for matmul rhs
NSPAN = NTOK // SPAN        # 8
NQB = S // P                # 16 q blocks per batch element
KCH = D // P                # 16 contraction chunks of 128 over d_model
FSL = DFFC // P             # 8 ffn slices of 128
F32 = mybir.dt.float32
BF16 = mybir.dt.bfloat16

_CACHE = {}
LAST_RESULT = None


# ---------------------------------------------------------------- program
def build_program():
    nc = bacc.Bacc(
        "TRN2", target_bir_lowering=False, debug=False, num_devices=NCORES
    )

    # -------- per-core I/O (same shapes on every core; data differs)
    xc = nc.dram_tensor("xc", [TOKC, D], F32, kind="ExternalInput").ap()
    alibi = nc.dram_tensor("alibi", [HPC, S, S], F32, kind="ExternalInput").ap()
    wq = nc.dram_tensor("wq", [D, HPC * HD], BF16, kind="ExternalInput").ap()
    wk = nc.dram_tensor("wk", [D, HPC * HD], BF16, kind="ExternalInput").ap()
    wv = nc.dram_tensor("wv", [D, HPC * HD], BF16, kind="ExternalInput").ap()
    bqkv = nc.dram_tensor("bqkv", [3 * HPC * HD], F32, kind="ExternalInput").ap()
    wo = nc.dram_tensor("wo", [HPC * HD, D], BF16, kind="ExternalInput").ap()
    w1 = nc.dram_tensor("w1", [D, DFFC], BF16, kind="ExternalInput").ap()
    b1 = nc.dram_tensor("b1", [DFFC], F32, kind="ExternalInput").ap()
    w2 = nc.dram_tensor("w2", [DFFC, D], BF16, kind="ExternalInput").ap()

    ffn_part = nc.dram_tensor(
        "ffn_part", [NTOK, D], BF16, kind="ExternalOutput"
    ).ap()
    h_part = nc.dram_tensor("h_part", [TOKC, D], F32, kind="ExternalOutput").ap()

    groups = [list(range(NCORES))]

    with tile.TileContext(nc) as tc:
        _build(tc, nc, xc, alibi, wq, wk, wv, bqkv, wo, w1, b1, w2,
               ffn_part, h_part, groups)

    nc.compile()
    return nc


def _layernorm_tiles(nc, pool, src_rows, dst_rows, n_tiles, h_out_rows=None,
                     rs_rows=None, tag=""):
    """LN over [n_tiles*128, D] rows: read f32 rows from `src_rows`
    (callable tile_idx -> DRAM AP), write normalized bf16 rows to
    `dst_rows`. If rs_rows is given, first add it (bf16 residual) to the
    input and also store the f32 sum to h_out_rows."""
    eps_t = pool.tile([P, 1], F32, tag=f"ln_eps{tag}", bufs=1)
    nc.vector.memset(eps_t[:], LN_EPS)
    for t in range(n_tiles):
        x_t = pool.tile([P, D], F32, tag=f"ln_x{tag}", name=f"ln_x{tag}_{t}")
        nc.gpsimd.dma_start(out=x_t[:], in_=src_rows(t))
        if rs_rows is not None:
            rs_t = pool.tile([P, D], BF16, tag=f"ln_rs{tag}",
                             name=f"ln_rs{tag}_{t}")
            nc.gpsimd.dma_start(out=rs_t[:], in_=rs_rows(t))
            nc.vector.tensor_add(out=x_t[:], in0=x_t[:], in1=rs_t[:])
            nc.gpsimd.dma_start(out=h_out_rows(t), in_=x_t[:])
        # stats: 4 chunks of 512 through bn_stats, then aggregate
        x_view = x_t[:].rearrange("p (c f) -> p c f", f=512)
        st = pool.tile([P, 4, 6], F32, tag=f"ln_st{tag}", name=f"ln_st{tag}_{t}")
        for c in range(4):
            nc.vector.bn_stats(out=st[:, c, :], in_=x_view[:, c, :])
        mv = pool.tile([P, 2], F32, tag=f"ln_mv{tag}", name=f"ln_mv{tag}_{t}")
        nc.vector.bn_aggr(out=mv[:], in_=st[:])
        # inv = 1/sqrt(var+eps)
        inv = pool.tile([P, 1], F32, tag=f"ln_inv{tag}", name=f"ln_inv{tag}_{t}")
        nc.scalar.activation(out=inv[:], in_=mv[:, 1:2],
                             func=mybir.ActivationFunctionType.Sqrt,
                             bias=eps_t[:], scale=1.0)
        nc.vector.reciprocal(out=inv[:], in_=inv[:])
        xh = pool.tile([P, D], BF16, tag=f"ln_xh{tag}", name=f"ln_xh{tag}_{t}")
        nc.vector.tensor_scalar(
            out=xh[:], in0=x_t[:], scalar1=mv[:, 0:1], scalar2=inv[:],
            op0=mybir.AluOpType.subtract, op1=mybir.AluOpType.mult)
        nc.gpsimd.dma_start(out=dst_rows(t), in_=xh[:])


def _build(tc, nc, xc, alibi, wq, wk, wv, bqkv, wo, w1, b1, w2,
           ffn_part, h_part, groups):
    import contextlib

    ctx = contextlib.ExitStack()
    with ctx:
        dram = ctx.enter_context(tc.tile_pool(name="dram", bufs=1, space="DRAM"))
        xhat_in = dram.tile([TOKC, D], BF16)
        xhat_ag = dram.tile([NTOK, D], BF16, addr_space="Shared")
        attn_part = dram.tile([NTOK, D], BF16)
        rs_out = dram.tile([TOKC, D], BF16)
        hn_in = dram.tile([TOKC, D], BF16)
        hn_ag = dram.tile([NTOK, D], BF16, addr_space="Shared")

        # persistent sbuf: qT/kT per head, v natural per batch (both heads)
        persist = ctx.enter_context(tc.tile_pool(name="persist", bufs=1))
        qT = [persist.tile([P, NTOK], BF16, name=f"qT{h}") for h in range(HPC)]
        kT = [persist.tile([P, NTOK], BF16, name=f"kT{h}") for h in range(HPC)]
        vnat = [persist.tile([P, NQB, HPC * HD], BF16, name=f"vnat{b}")
                for b in range(B)]

        # small weights / constants (loaded once, early)
        wpool = ctx.enter_context(tc.tile_pool(name="weights", bufs=1))
        bqkv_sb = wpool.tile([P, 3 * HPC], F32)
        nc.gpsimd.dma_start(out=bqkv_sb[:],
                            in_=bqkv.rearrange("(a p) -> p a", p=P))
        bv_bc = wpool.tile([P, HPC * HD], F32)
        _bv = bqkv[2 * HPC * HD:3 * HPC * HD]
        nc.gpsimd.dma_start(
            out=bv_bc[:],
            in_=bass.AP(tensor=_bv.tensor, offset=_bv.offset,
                        ap=[[0, P]] + [list(a) for a in _bv.ap]))
        wo_sb = wpool.tile([P, HPC, D], BF16)
        nc.gpsimd.dma_start(out=wo_sb[:], in_=wo.rearrange("(h p) o -> p h o", p=P))
        ones_bf = wpool.tile([P, 1], BF16)
        nc.vector.memset(ones_bf[:], 1.0)
        ones_row = wpool.tile([1, P], F32)
        nc.vector.memset(ones_row[:], 1.0)

        # ---------------- phase A: LN1 on own token slice, AllGather
        with tc.tile_pool(name="ln1", bufs=2) as ln1p:
            _layernorm_tiles(
                nc, ln1p,
                src_rows=lambda t: xc[t * P:(t + 1) * P, :],
                dst_rows=lambda t: xhat_in[t * P:(t + 1) * P, :],
                n_tiles=TOKC // P, tag="1")
        nc.gpsimd.collective_compute(
            "AllGather", mybir.AluOpType.bypass, replica_groups=groups,
            ins=[xhat_in.opt()], outs=[xhat_ag.opt()])

        # ---------------- phase B: QKV projections
        # qT/kT come out transposed [head_dim, tok]; v comes out natural
        # [tok, head_dim] (needed as AV lhsT), so no extra transposes.
        WTOK = 2 * SPAN  # tokens per transposed x tile
        with tc.tile_pool(name="qkvw", bufs=1) as qwp, \
             tc.tile_pool(name="qkv", bufs=2) as qkvp, \
             tc.tile_pool(name="qkv_ps", bufs=4, space="PSUM") as qkvps, \
             tc.tile_pool(name="v_ps", bufs=2, space="PSUM") as vps:
            wq_sb = qwp.tile([P, KCH, HPC * HD], BF16)
            wk_sb = qwp.tile([P, KCH, HPC * HD], BF16)
            wv_sb = qwp.tile([P, KCH, HPC * HD], BF16)
            nc.gpsimd.dma_start(out=wq_sb[:],
                                in_=wq.rearrange("(c p) o -> p c o", p=P))
            nc.gpsimd.dma_start(out=wk_sb[:],
                                in_=wk.rearrange("(c p) o -> p c o", p=P))
            nc.gpsimd.dma_start(out=wv_sb[:],
                                in_=wv.rearrange("(c p) o -> p c o", p=P))
            for wt in range(NTOK // WTOK):
                xT = qkvp.tile([P, KCH, WTOK], BF16, tag="xT")
                for kc in range(KCH):
                    nc.sync.dma_start(
                        out=xT[:, kc, :],
                        in_=xhat_ag[wt * WTOK:(wt + 1) * WTOK,
                                    kc * P:(kc + 1) * P],
                        transpose=True)
                for half in range(WTOK // SPAN):
                    tok0 = wt * WTOK + half * SPAN
                    for wi, (w_sb, outs) in enumerate(
                            ((wq_sb, qT), (wk_sb, kT))):
                        for h in range(HPC):
                            ps = qkvps.tile([P, SPAN], F32, tag="ps")
                            for kc in range(KCH):
                                nc.tensor.matmul(
                                    ps[:],
                                    lhsT=w_sb[:, kc, h * HD:(h + 1) * HD],
                                    rhs=xT[:, kc,
                                           half * SPAN:(half + 1) * SPAN],
                                    start=(kc == 0), stop=(kc == KCH - 1))
                            col = wi * HPC + h
                            nc.scalar.activation(
                                out=outs[h][:, tok0:tok0 + SPAN], in_=ps[:],
                                func=mybir.ActivationFunctionType.Identity,
                                bias=bqkv_sb[:, col:col + 1], scale=1.0)
                # v natural: one [128-tok, 256] psum per token block
                for tb in range(WTOK // P):
                    gtok = wt * WTOK + tb * P
                    b, j = divmod(gtok, S)
                    j //= P
                    vp = vps.tile([P, HPC * HD], F32, tag="vps")
                    for kc in range(KCH):
                        nc.tensor.matmul(
                            vp[:], lhsT=xT[:, kc, tb * P:(tb + 1) * P],
                            rhs=wv_sb[:, kc, :],
                            start=(kc == 0), stop=(kc == KCH - 1))
                    nc.vector.scalar_tensor_tensor(
                        out=vnat[b][:, j, :], in0=vp[:], scalar=1.0,
                        in1=bv_bc[:], op0=mybir.AluOpType.mult,
                        op1=mybir.AluOpType.add)

        # ---------------- phase C: attention + W_o partials
        # Scores are computed TRANSPOSED: scT[k, q] = kT.T @ qT, the alibi
        # input is pre-transposed (+causal mask) on the host, so exp gives
        # E^T [k, q] directly = the AV matmul rhs. No on-device transposes.
        # Softmax denominators via a ones-row matmul on PE; the reciprocal
        # is partition-broadcast with a tiny DMA.
        NSP = S // SPAN  # q spans per sequence (4)
        with tc.tile_pool(name="att", bufs=3) as ap_, \
             tc.tile_pool(name="att_sm", bufs=4) as smp, \
             tc.tile_pool(name="avt", bufs=8) as avtp, \
             tc.tile_pool(name="att_ps", bufs=2, space="PSUM") as aps, \
             tc.tile_pool(name="av_ps", bufs=2, space="PSUM") as avps, \
             tc.tile_pool(name="den_ps", bufs=2, space="PSUM") as denps, \
             tc.tile_pool(name="wo_ps", bufs=2, space="PSUM") as wops:
            for m in range(NSP):
                nkb = 4 * (m + 1)            # causal: k blocks 0..4m+3
                avT = [[None] * B for _ in range(HPC)]
                for h in range(HPC):
                    av_ps = [avps.tile([P, SPAN], F32, tag="avps",
                                       name=f"avp_{m}_{h}_{b}")
                             for b in range(B)]
                    den_ps = [denps.tile([1, SPAN], F32, tag="denps",
                                         name=f"den_{m}_{h}_{b}")
                              for b in range(B)]
                    for j in range(nkb):
                        al_t = ap_.tile([P, SPAN], F32, tag="alibi",
                                        name=f"al_{m}_{h}_{j}")
                        nc.gpsimd.dma_start(
                            out=al_t[:],
                            in_=alibi[h, j * P:(j + 1) * P,
                                      m * SPAN:(m + 1) * SPAN])
                        for b in range(B):
                            toff = b * S
                            ps = aps.tile([P, SPAN], F32, tag="scps",
                                          name=f"sc_{m}_{h}_{j}_{b}")
                            nc.tensor.matmul(
                                ps[:],
                                lhsT=kT[h][:, toff + j * P:toff + (j + 1) * P],
                                rhs=qT[h][:, toff + m * SPAN:
                                          toff + (m + 1) * SPAN],
                                start=True, stop=True)
                            s_t = smp.tile([P, SPAN], F32, tag="s",
                                           name=f"s_{m}_{h}_{j}_{b}")
                            nc.vector.scalar_tensor_tensor(
                                out=s_t[:], in0=ps[:], scalar=1.0,
                                in1=al_t[:], op0=mybir.AluOpType.mult,
                                op1=mybir.AluOpType.add)
                            ET_t = smp.tile([P, SPAN], BF16, tag="ET",
                                            name=f"ET_{m}_{h}_{j}_{b}")
                            nc.scalar.activation(
                                out=ET_t[:], in_=s_t[:],
                                func=mybir.ActivationFunctionType.Exp)
                            nc.tensor.matmul(
                                av_ps[b][:],
                                lhsT=vnat[b][:, j, h * HD:(h + 1) * HD],
                                rhs=ET_t[:],
                                start=(j == 0), stop=(j == nkb - 1),
                                skip_group_check=True)
                            nc.tensor.matmul(
                                den_ps[b][:], lhsT=ones_bf[:], rhs=ET_t[:],
                                start=(j == 0), stop=(j == nkb - 1),
                                skip_group_check=True)
                    for b in range(B):
                        rec = smp.tile([1, SPAN], F32, tag="rec",
                                       name=f"rec_{m}_{h}_{b}")
                        nc.vector.reciprocal(out=rec[:], in_=den_ps[b][:])
                        bc_ps = aps.tile([P, SPAN], F32, tag="scps",
                                         name=f"bcps_{m}_{h}_{b}")
                        nc.tensor.matmul(bc_ps[:], lhsT=ones_row[:],
                                         rhs=rec[:], start=True, stop=True)
                        rec_bc = smp.tile([P, SPAN], F32, tag="recbc",
                                          name=f"recbc_{m}_{h}_{b}")
                        nc.scalar.copy(out=rec_bc[:], in_=bc_ps[:])
                        avT_t = avtp.tile([P, SPAN], BF16, tag="avT",
                                          name=f"avT_{m}_{h}_{b}")
                        nc.vector.scalar_tensor_tensor(
                            out=avT_t[:], in0=av_ps[b][:], scalar=1.0,
                            in1=rec_bc[:], op0=mybir.AluOpType.mult,
                            op1=mybir.AluOpType.mult)
                        avT[h][b] = avT_t
                # W_o for this q span (accumulate over local heads)
                for b in range(B):
                    for qb in range(SPAN // P):
                        for dsp in range(D // SPAN):
                            ps = wops.tile([P, SPAN], F32, tag="wops",
                                           name=f"wo_{m}_{b}_{qb}_{dsp}")
                            for h in range(HPC):
                                nc.tensor.matmul(
                                    ps[:],
                                    lhsT=avT[h][b][:, qb * P:(qb + 1) * P],
                                    rhs=wo_sb[:, h,
                                              dsp * SPAN:(dsp + 1) * SPAN],
                                    start=(h == 0), stop=(h == HPC - 1))
                            o_sb = smp.tile([P, SPAN], BF16, tag="wo_o",
                                            name=f"woo_{m}_{b}_{qb}_{dsp}")
                            nc.any.tensor_copy(out=o_sb[:], in_=ps[:])
                            row = b * S + m * SPAN + qb * P
                            nc.gpsimd.dma_start(
                                out=attn_part[row:row + P,
                                              dsp * SPAN:(dsp + 1) * SPAN],
                                in_=o_sb[:])

        nc.gpsimd.collective_compute(
            "ReduceScatter", mybir.AluOpType.add, replica_groups=groups,
            ins=[attn_part.opt()], outs=[rs_out.opt()])

        # ---------------- phase D: h = x + attn, LN2, AllGather
        with tc.tile_pool(name="ln2", bufs=2) as ln2p:
            _layernorm_tiles(
                nc, ln2p,
                src_rows=lambda t: xc[t * P:(t + 1) * P, :],
                dst_rows=lambda t: hn_in[t * P:(t + 1) * P, :],
                n_tiles=TOKC // P,
                h_out_rows=lambda t: h_part[t * P:(t + 1) * P, :],
                rs_rows=lambda t: rs_out[t * P:(t + 1) * P, :], tag="2")
        nc.gpsimd.collective_compute(
            "AllGather", mybir.AluOpType.bypass, replica_groups=groups,
            ins=[hn_in.opt()], outs=[hn_ag.opt()])

        # ---------------- phase E: FFN
        with tc.tile_pool(name="ffnw", bufs=1) as fwp, \
             tc.tile_pool(name="ffn", bufs=2) as ffnp, \
             tc.tile_pool(name="ffn_ps", bufs=4, space="PSUM") as fps:
            w1_sb = fwp.tile([P, KCH, DFFC], BF16)
            nc.gpsimd.dma_start(out=w1_sb[:],
                                in_=w1.rearrange("(c p) f -> p c f", p=P))
            b1_sb = fwp.tile([P, FSL], F32)
            nc.gpsimd.dma_start(out=b1_sb[:],
                                in_=b1.rearrange("(s p) -> p s", p=P))
            w2_sb = fwp.tile([P, FSL, D], BF16)
            nc.gpsimd.dma_start(out=w2_sb[:],
                                in_=w2.rearrange("(c p) o -> p c o", p=P))
            WTOK2 = 2 * SPAN
            for wt in range(NTOK // WTOK2):
                hT = ffnp.tile([P, KCH, WTOK2], BF16, tag="hT")
                for kc in range(KCH):
                    nc.sync.dma_start(
                        out=hT[:, kc, :],
                        in_=hn_ag[wt * WTOK2:(wt + 1) * WTOK2,
                                  kc * P:(kc + 1) * P],
                        transpose=True)
                for half in range(WTOK2 // SPAN):
                    hsl = slice(half * SPAN, (half + 1) * SPAN)
                    g1 = ffnp.tile([P, FSL, SPAN], BF16, tag="g1")
                    for s in range(FSL):
                        ps = fps.tile([P, SPAN], F32, tag="f1ps")
                        for kc in range(KCH):
                            nc.tensor.matmul(
                                ps[:], lhsT=w1_sb[:, kc, s * P:(s + 1) * P],
                                rhs=hT[:, kc, hsl],
                                start=(kc == 0), stop=(kc == KCH - 1))
                        nc.scalar.activation(
                            out=g1[:, s, :], in_=ps[:],
                            func=mybir.ActivationFunctionType.Gelu,
                            bias=b1_sb[:, s:s + 1], scale=1.0)
                    for tb in range(SPAN // P):
                        for dsp in range(D // SPAN):
                            ps2 = fps.tile([P, SPAN], F32, tag="f2ps")
                            for s in range(FSL):
                                nc.tensor.matmul(
                                    ps2[:], lhsT=g1[:, s, tb * P:(tb + 1) * P],
                                    rhs=w2_sb[:, s,
                                              dsp * SPAN:(dsp + 1) * SPAN],
                                    start=(s == 0), stop=(s == FSL - 1))
                            o_sb = ffnp.tile([P, SPAN], BF16, tag="fo")
                            nc.any.tensor_copy(out=o_sb[:], in_=ps2[:])
                            row = wt * WTOK2 + half * SPAN + tb * P
                            nc.gpsimd.dma_start(
                                out=ffn_part[row:row + P,
                                             dsp * SPAN:(dsp + 1) * SPAN],
                                in_=o_sb[:])


# ---------------------------------------------------------------- host side
def _prep_inputs(x, alibi_bias, W_q, W_k, W_v, W_o, ln1_g, ln1_b, ln2_g,
                 ln2_b, ffn_w1, ffn_b1, ffn_w2, ffn_b2):
    f32 = np.float32
    x = np.ascontiguousarray(np.asarray(x, f32).reshape(NTOK, D))
    inv_sqrt_hd = f32(1.0 / math.sqrt(HD))
    ln1_g = np.asarray(ln1_g, f32)
    ln1_b = np.asarray(ln1_b, f32)
    ln2_g = np.asarray(ln2_g, f32)
    ln2_b = np.asarray(ln2_b, f32)

    wq_f = (ln1_g[:, None] * np.asarray(W_q, f32)) * inv_sqrt_hd
    bq = (ln1_b @ np.asarray(W_q, f32)) * inv_sqrt_hd
    wk_f = ln1_g[:, None] * np.asarray(W_k, f32)
    bk = ln1_b @ np.asarray(W_k, f32)
    wv_f = ln1_g[:, None] * np.asarray(W_v, f32)
    bv = ln1_b @ np.asarray(W_v, f32)
    w1_f = ln2_g[:, None] * np.asarray(ffn_w1, f32)
    b1_f = ln2_b @ np.asarray(ffn_w1, f32) + np.asarray(ffn_b1, f32)

    # alibi with causal mask folded in, TRANSPOSED to [head, k, q]
    al = np.asarray(alibi_bias, f32).copy()
    iu = np.triu_indices(S, k=1)
    al[:, iu[0], iu[1]] = NEG
    al = np.ascontiguousarray(al.transpose(0, 2, 1))

    W_o = np.asarray(W_o, f32)
    w2 = np.asarray(ffn_w2, f32)

    in_maps = []
    for c in range(NCORES):
        hs = slice(c * HPC * HD, (c + 1) * HPC * HD)     # head-dim slice
        fs = slice(c * DFFC, (c + 1) * DFFC)             # ffn slice
        ts_ = slice(c * TOKC, (c + 1) * TOKC)            # token slice
        bqkv_c = np.concatenate([bq[hs], bk[hs], bv[hs]]).astype(f32)
        in_maps.append({
            "xc": np.ascontiguousarray(x[ts_]),
            "alibi": np.ascontiguousarray(al[c * HPC:(c + 1) * HPC]),
            "wq": np.ascontiguousarray(wq_f[:, hs].astype(np_bf16)),
            "wk": np.ascontiguousarray(wk_f[:, hs].astype(np_bf16)),
            "wv": np.ascontiguousarray(wv_f[:, hs].astype(np_bf16)),
            "bqkv": bqkv_c,
            "wo": np.ascontiguousarray(W_o[hs, :].astype(np_bf16)),
            "w1": np.ascontiguousarray(w1_f[:, fs].astype(np_bf16)),
            "b1": np.ascontiguousarray(b1_f[fs]),
            "w2": np.ascontiguousarray(w2[fs, :].astype(np_bf16)),
        })
    return in_maps


def kernel(x, alibi_bias, W_q, W_k, W_v, W_o, ln1_g, ln1_b, ln2_g, ln2_b,
           ffn_w1, ffn_b1, ffn_w2, ffn_b2, *, _trace=False, _tmpdir=None):
    global LAST_RESULT
    if "nc" not in _CACHE:
        _CACHE["nc"] = build_program()
    nc = _CACHE["nc"]

    in_maps = _prep_inputs(x, alibi_bias, W_q, W_k, W_v, W_o, ln1_g, ln1_b,
                           ln2_g, ln2_b, ffn_w1, ffn_b1, ffn_w2, ffn_b2)

    res = run_bass_kernel_spmd(
        nc, in_maps, core_ids=list(range(NCORES)),
        trace=_trace, tmpdir=_tmpdir)
    LAST_RESULT = res

    out = np.zeros((NTOK, D), np.float32)
    for c in range(NCORES):
        out += np.asarray(res.results[c]["ffn_part"], np.float32)
    for c in range(NCORES):
        out[c * TOKC:(c + 1) * TOKC] += np.asarray(res.results[c]["h_part"])
    out += np.asarray(ffn_b2, np.float32)[None, :]
    return out.reshape(B, S, D)
